# revision 25
# baseline (speedup 1.0000x reference)
"""Trainium2 Bass kernel for nn_KANCouplingNet (3-layer KAN MLP, widths 12-64-64-24).

Math: each KAN layer is y = silu(x) @ sb + B(x) contracted with coef*ss.  On the
uniform grid every basis function is a shifted cardinal spline M(s - g),
s = x/0.4 + 5.5.  M is computed in ONE 8-stage custom DVE instruction per page
via the exact identity (w = |s - g - 2|):
    6*M = min(4 - 3w^2*(2-w), relu(2-w)^3)
with the helper stream 3w^2 (Src1) produced off the critical DVE path by the
Scalar engine (Square activation), a 4-op paged DVE square op, and GpSimd
(layer 0, SBUF-sourced).  Everything runs in h-space (h = raw layer input,
read straight from PSUM): w = |h - a|, a = (g - 3.5)*0.4, thresholds 0.8 /
4*0.4^3; the S^3/6 basis scale folds into the matmul weights.  One feature
per (channel, shift) - half the DVE elements and half the matmuls of the
two-relu-cube fold.

Features/silu/weights/squares are bf16 (features bounded <= 0.256, coefs
~0.1, and the P-branch only matters for w < 0.4 where 3w^2*c1 <= 0.19, so
bf16 Src1 is safe; measured 1.4e-3 end-to-end rel err): the PE streams bf16
at 1 col/cycle vs fp32r's 2-pass.  The pipeline is software-skewed (input
prefetch + gpsimd L0 squares run 2 tiles ahead; the L0 spline sits last in
the per-iteration DVE stream so it never head-of-line-blocks the strict-FIFO
vector queue), PSUM tags are triple-buffered (3+3+2 banks), and the 8-way
x-replication is a single stride-0 DRAM-read DMA (the Sync engine's serial
DIRECT2D issue at ~750ns each was a hidden bottleneck).

Sharding: pure data parallel over the batch dim (32 batches -> 4 per core).
"""
import dataclasses
import math

import numpy as np

import concourse.bacc as bacc
import concourse.bass as bass
import concourse.mybir as mybir
import concourse.tile as tile
from concourse.bass_utils import run_bass_kernel_spmd

FP = mybir.dt.float32
BF = mybir.dt.bfloat16
AFT = mybir.ActivationFunctionType
ALU = mybir.AluOpType

N_CORES = 8
B_PER_CORE = 4          # 32 batches / 8 cores
HW = 64 * 64            # 4096 pixels per batch image
NT = 512                # pixel tile
TILES_PER_B = HW // NT  # 4
MM_N = 512              # matmul moving free dim (psum bank caps output at 512 fp32)
H_GRID = 0.4
S_SCALE = 1.0 / H_GRID          # 2.5
TH_H = 2.0 * H_GRID             # spline half-width in h-space: 0.8
PC_H = 4.0 * H_GRID ** 3        # P-branch constant in h-space: 0.256
FSCALE = S_SCALE ** 3 / 6.0     # feature -> basis scale folded into weights
SQRT3 = math.sqrt(3.0)

# square-page engine map per 64-ch layer: pages 0..3 -> 'dve' (one paged call
# for the leading run of dve pages) or 'sc' (scalar Square)
L1_SQ = ("sc", "sc", "sc", "sc")
L2_SQ = ("sc", "sc", "sc", "sc")

_OPS = {}
_CACHE = {}


def _register_ops():
    """Direct-M spline consumer + paged 3(x-a)^2 producer (idempotent)."""
    if _OPS:
        return _OPS
    from concourse import dve_ops
    from concourse.dve_spec import (AluOp, Bin, C0, C1, C2, PageIdx, Spec,
                                    Src0, Src1, _has_src1 as has_src1, lower,
                                    minn, relu, sq)
    from concourse.dve_uop import DveOpSpec

    def _add(name, spec, subdim):
        for op in dve_ops.OPS:
            if op.name == name:
                _OPS[name] = op
                return
        row = dve_ops._CUSTOM_DVE_ROW_BASE + len(dve_ops.OPS)
        shas = {}
        for ver in ("v3", "v4"):
            tmp = DveOpSpec(name=name, opcode=row, uops=lower(spec, ver=ver),
                            rd1_en=has_src1(spec))
            shas[ver] = tmp.sha(ver)
        op = dve_ops.DveOp(name, spec, subdim=subdim, uops_sha=shas)
        dve_ops.OPS.append(op)
        dve_ops._SUB_OPCODE_FOR_NAME[op.name] = row
        dve_ops.CUSTOM_DVE_SPECS[op.name] = spec
        _OPS[name] = op

    # ---- MSPLINE: out = min(imm2 - in1*c1, relu(c1)^3), c1 = s1 - |in0 - s0|
    w = Bin(AluOp.ABSOLUTE_DIFF, Src0, C0)
    c1 = Bin(AluOp.SUBTRACT, C1, w)
    body = minn(Bin(AluOp.SUBTRACT, C2, Src1 * c1), sq(c1) * relu(c1))

    def _mspline_ref(in0, in1, s0, s1, imm2):
        in0 = np.asarray(in0, np.float32)
        in1 = np.asarray(in1, np.float32)
        s0v = np.asarray(s0, np.float32).reshape((-1,) + (1,) * (in0.ndim - 1))
        cc = np.float32(s1) - np.abs(in0 - s0v)
        return np.minimum(np.float32(imm2) - in1 * cc,
                          np.maximum(cc, 0.0) ** 3).astype(np.float32)

    _add("MSPLINE_ANT", Spec(body=body, reference=_mspline_ref), False)

    # ---- SQ3PG: out[p,s,n] = imm2 * (in0[p,s,n] - (s0[p] + s*s1))^2
    d = Bin(AluOp.SUBTRACT, Src0, PageIdx(C0, C1))
    body2 = sq(d) * C2

    def _sq3_ref(in0, in1, s0, s1, imm2):
        in0 = np.asarray(in0, np.float32)
        if in0.ndim == 3:
            base = np.asarray(s0, np.float32).reshape(-1, 1, 1) \
                + np.arange(in0.shape[1], dtype=np.float32).reshape(1, -1, 1) * np.float32(s1)
        else:
            base = np.asarray(s0, np.float32).reshape(-1, 1)
        dd = in0 - base
        return (dd * dd * np.float32(imm2)).astype(np.float32)

    _add("SQ3PG_ANT", Spec(body=body2, reference=_sq3_ref), True)
    return _OPS


def _paged(ap: bass.AP, s: int) -> bass.AP:
    """View a flat [P, N] AP as [P, s, N] with a step-0 page dim."""
    return dataclasses.replace(ap, ap=[ap.ap[0], [0, s], ap.ap[1]])


def _centers():
    a = lambda g: (g - 3.5) * H_GRID
    c0 = np.array([a(p // 12) for p in range(96)], np.float32)
    c1 = np.zeros((4, 128), np.float32)
    for page in range(4):
        for q in range(128):
            c1[page, q] = a(4 * (q // 64) + page)
    return c0, c1


def _host_weights(coef, sb, ss, din, dout):
    """Stationary weights, direct-M layout, FSCALE folded; cols duplicated
    when dout == 64 so PSUM holds h in both partition halves."""
    cp = coef.astype(np.float64) * ss.astype(np.float64)[:, :, None] * FSCALE
    mcols = 128 if dout == 64 else dout
    if din == 12:
        lhs = np.zeros((96, mcols), np.float32)
        for g in range(8):
            for i in range(12):
                for o in range(dout):
                    v = cp[i, o, g]
                    lhs[g * 12 + i, o] = v
                    if mcols == 128:
                        lhs[g * 12 + i, o + 64] = v
    else:
        lhs = np.zeros((4, 128, mcols), np.float32)
        for page in range(4):
            for q in range(128):
                i, g = q % 64, 4 * (q // 64) + page
                for o in range(dout):
                    v = cp[i, o, g]
                    lhs[page, q, o] = v
                    if mcols == 128:
                        lhs[page, q, o + 64] = v
    base = np.zeros((din if din == 12 else 64, mcols), np.float32)
    base[:, :dout] = sb
    if mcols == 128:
        base[:, 64:64 + dout] = sb
    return lhs, base


def _build(trace_sim=False):
    ops = _register_ops()
    msp = ops["MSPLINE_ANT"]
    sqp3 = ops["SQ3PG_ANT"]
    nc = bacc.Bacc("TRN2", target_bir_lowering=False, debug=False,
                   enable_asserts=False, num_devices=N_CORES)

    x_d = nc.dram_tensor("x_in", [B_PER_CORE, 12, HW], FP, kind="ExternalInput").ap()
    out_d = nc.dram_tensor("y_out", [B_PER_CORE, 24, HW], FP, kind="ExternalOutput").ap()
    w0_d = nc.dram_tensor("w0", [108, 128], BF, kind="ExternalInput").ap()
    w1_d = nc.dram_tensor("w1", [4, 128, 128], BF, kind="ExternalInput").ap()
    b1_d = nc.dram_tensor("b1", [64, 128], BF, kind="ExternalInput").ap()
    w2_d = nc.dram_tensor("w2", [4, 128, 24], BF, kind="ExternalInput").ap()
    b2_d = nc.dram_tensor("b2", [64, 24], BF, kind="ExternalInput").ap()
    # cols: 0 cen L0, 1 -sqrt3*cen L0, 2..5 cen L1/L2 page, 6..9 -sqrt3*cen page
    cns_d = nc.dram_tensor("cns", [128, 10], FP, kind="ExternalInput").ap()

    def n_dve_pages(sq_map):
        n = 0
        for s in sq_map:
            if s != "dve":
                break
            n += 1
        return n

    with tile.TileContext(nc, trace_sim=trace_sim) as tc:
        with (
            tc.tile_pool(name="consts", bufs=1) as cp,
            tc.tile_pool(name="xin", bufs=3) as xp,
            tc.tile_pool(name="xrep", bufs=3) as xrp,
            tc.tile_pool(name="sq", bufs=4) as sqpool,
            tc.tile_pool(name="sil", bufs=4) as silp,
            tc.tile_pool(name="feat", bufs=4) as fpool,
            tc.tile_pool(name="yout", bufs=3) as yp,
            tc.tile_pool(name="ps1", bufs=3, space="PSUM") as pp1,
            tc.tile_pool(name="ps2", bufs=3, space="PSUM") as pp2,
            tc.tile_pool(name="ps3", bufs=2, space="PSUM") as pp3,
        ):
            cns = cp.tile([128, 10], FP, tag="cns")
            nc.sync.dma_start(cns[:], cns_d[:])
            # dummy spline op: absorbs the DVE custom-op table load during the
            # DMA ramp so the first real feature call isn't delayed by it
            warm = cp.tile([128, 1], FP, tag="warm")
            nc.vector._custom_dve(
                _OPS["MSPLINE_ANT"], out=warm[:], in0=cns[:, 0:1],
                in1=cns[:, 1:2], s0=0.0, s1=1.0, imm2=1.0)
            w0 = cp.tile([108, 128], BF, tag="w0")
            w1 = [cp.tile([128, 128], BF, tag=f"w1_{g}", name=f"w1_{g}") for g in range(4)]
            w2 = [cp.tile([128, 24], BF, tag=f"w2_{g}", name=f"w2_{g}") for g in range(4)]
            b1 = cp.tile([64, 128], BF, tag="b1")
            b2 = cp.tile([64, 24], BF, tag="b2")

            def load_weights():
                nc.sync.dma_start(w0[:], w0_d[:])
                for g in range(4):
                    nc.sync.dma_start(w1[g][:], w1_d[g])
                    nc.sync.dma_start(w2[g][:], w2_d[g])
                nc.sync.dma_start(b1[:], b1_d[:])
                nc.sync.dma_start(b2[:], b2_d[:])

            def layer_elem(lname, ps, f, sq_map):
                """Producers + consumers for a 64-ch layer from PSUM tile ps
                into bf16 feature tile f [128, 4*NT]."""
                ndve = n_dve_pages(sq_map)
                if ndve:
                    sqd = sqpool.tile([128, ndve * NT], BF, tag=f"sq_{lname}_d",
                                      name=f"sq_{lname}_d")
                    nc.vector._custom_dve(
                        sqp3, out=dataclasses.replace(
                            sqd[:], ap=[sqd[:].ap[0], [NT, ndve], [1, NT]]),
                        in0=_paged(ps[:], ndve),
                        s0=cns[:, 2:3], s1=H_GRID, imm2=3.0)
                sqs = {}
                for page in range(4):
                    if page < ndve:
                        sqs[page] = sqd[:, bass.ts(page, NT)]
                    else:
                        t = sqpool.tile([128, NT], BF, tag=f"sq_{lname}_{page}",
                                        name=f"sq_{lname}_{page}")
                        nc.scalar.activation(t[:], ps[:], AFT.Square,
                                             bias=cns[:, 6 + page:7 + page],
                                             scale=SQRT3)
                        sqs[page] = t[:]
                for page in range(4):
                    nc.vector._custom_dve(msp, out=f[:, bass.ts(page, NT)],
                                          in0=ps[:], in1=sqs[page],
                                          s0=cns[:, 2 + page:3 + page],
                                          s1=TH_H, imm2=PC_H)

            def mm(ps, lhsT, rhs, start, stop):
                if MM_N == NT:
                    nc.tensor.matmul(ps[:], lhsT, rhs, start=start, stop=stop)
                else:
                    for h in range(NT // MM_N):
                        nc.tensor.matmul(ps[:, bass.ts(h, MM_N)], lhsT,
                                         rhs[:, bass.ts(h, MM_N)],
                                         start=start, stop=stop,
                                         skip_group_check=True)

            NTILES = B_PER_CORE * TILES_PER_B
            st_ps1, st_ps2, st_pre = {}, {}, {}

            def stage_a0(idx):
                """Prefetch: DMA in + replicate + gpsimd L0 squares + sil0."""
                b, ti = divmod(idx, TILES_PER_B)
                cols = bass.ts(ti, NT)
                xt = xp.tile([12, NT], FP, tag="xt", name="xt")
                nc.sync.dma_start(xt[:], x_d[b, :, cols])
                xr = xrp.tile([96, NT], FP, tag="xr", name="xr")
                xin = x_d[b, :, cols]
                xin8 = dataclasses.replace(xin, ap=[[0, 8]] + list(xin.ap))
                nc.sync.dma_start(xr[:], xin8)
                t0 = sqpool.tile([96, NT], FP, tag="t0", name="t0")
                nc.gpsimd.tensor_scalar(t0[:], xr[:], SQRT3,
                                        cns[0:96, 1:2], ALU.mult, ALU.add)
                sq0 = sqpool.tile([96, NT], BF, tag="sq0", name="sq0")
                nc.gpsimd.tensor_tensor(sq0[:], t0[:], t0[:], ALU.mult)
                f0 = fpool.tile([108, NT], BF, tag="f0", name="f0")
                nc.scalar.activation(f0[96:108, :], xt[:], AFT.Silu)
                st_pre[idx] = (xr, sq0, f0)

            def stage_a1(idx):
                xr, sq0, f0 = st_pre.pop(idx)
                nc.vector._custom_dve(msp, out=f0[0:96, :], in0=xr[:],
                                      in1=sq0[:], s0=cns[0:96, 0:1],
                                      s1=TH_H, imm2=PC_H)
                ps1 = pp1.tile([128, NT], FP, tag="ps1", name="ps1")
                mm(ps1, w0[:], f0[:], True, True)
                st_ps1[idx] = ps1

            def stage_b(idx):
                ps1 = st_ps1.pop(idx)
                sil1 = silp.tile([64, NT], BF, tag="sil1", name="sil1")
                nc.scalar.activation(sil1[:], ps1[0:64, :], AFT.Silu)
                f1 = fpool.tile([128, 4 * NT], BF, tag="f1", name="f1")
                layer_elem("l1", ps1, f1, L1_SQ)
                ps2 = pp2.tile([128, NT], FP, tag="ps2", name="ps2")
                mm(ps2, b1[:], sil1[:], True, False)
                for page in range(4):
                    mm(ps2, w1[page][:], f1[:, bass.ts(page, NT)],
                       False, page == 3)
                st_ps2[idx] = ps2

            def stage_c(idx):
                b, ti = divmod(idx, TILES_PER_B)
                cols = bass.ts(ti, NT)
                ps2 = st_ps2.pop(idx)
                sil2 = silp.tile([64, NT], BF, tag="sil2", name="sil2")
                nc.scalar.activation(sil2[:], ps2[0:64, :], AFT.Silu)
                f2 = fpool.tile([128, 4 * NT], BF, tag="f2", name="f2")
                layer_elem("l2", ps2, f2, L2_SQ)
                ps3 = pp3.tile([24, NT], FP, tag="ps3", name="ps3")
                mm(ps3, b2[:], sil2[:], True, False)
                for page in range(4):
                    mm(ps3, w2[page][:], f2[:, bass.ts(page, NT)],
                       False, page == 3)
                yt = yp.tile([24, NT], FP, tag="yt", name="yt")
                nc.vector.tensor_copy(yt[:], ps3[:])
                nc.sync.dma_start(out_d[b, :, cols], yt[:])

            # skew: A0 prefetches 2 iterations ahead of A1's DVE/L0-MM; the
            # f0 spline sits last in the per-iteration DVE stream so it never
            # head-of-line-blocks ready L1/L2 feature work.
            stage_a0(0)
            stage_a0(1)
            load_weights()
            stage_a1(0)
            for idx in range(2, NTILES + 3):
                if idx < NTILES:
                    stage_a0(idx)
                if 2 <= idx <= NTILES + 1:
                    stage_b(idx - 2)
                if 3 <= idx <= NTILES + 2:
                    stage_c(idx - 3)
                if 1 <= idx <= NTILES:
                    stage_a1(idx - 1)

    nc.compile()
    return nc


def _in_maps(x):
    consts = _CACHE["consts"]
    x = np.asarray(x, np.float32).reshape(32, 12, HW)
    maps = []
    for c in range(N_CORES):
        m = dict(consts)
        m["x_in"] = np.ascontiguousarray(x[c * B_PER_CORE:(c + 1) * B_PER_CORE])
        maps.append(m)
    return maps


def kernel(x, grid0, coef0, sb0, ss0, grid1, coef1, sb1, ss1, grid2, coef2, sb2, ss2):
    import ml_dtypes
    bf16 = ml_dtypes.bfloat16
    if "nc" not in _CACHE:
        _CACHE["nc"] = _build()
    nc = _CACHE["nc"]

    w0f, b0 = _host_weights(np.asarray(coef0, np.float32), np.asarray(sb0, np.float32),
                            np.asarray(ss0, np.float32), 12, 64)
    w1f, b1 = _host_weights(np.asarray(coef1, np.float32), np.asarray(sb1, np.float32),
                            np.asarray(ss1, np.float32), 64, 64)
    w2f, b2 = _host_weights(np.asarray(coef2, np.float32), np.asarray(sb2, np.float32),
                            np.asarray(ss2, np.float32), 64, 24)
    w0 = np.concatenate([w0f, b0], axis=0)

    c0, c1 = _centers()
    cns = np.zeros((128, 10), np.float32)
    cns[0:96, 0] = c0
    cns[0:96, 1] = -SQRT3 * c0
    for page in range(4):
        cns[:, 2 + page] = c1[page]
        cns[:, 6 + page] = -SQRT3 * c1[page]

    _CACHE["consts"] = {
        "w0": w0.astype(bf16), "w1": w1f.astype(bf16), "b1": b1.astype(bf16),
        "w2": w2f.astype(bf16), "b2": b2.astype(bf16), "cns": cns,
    }
    maps = _in_maps(x)
    res = run_bass_kernel_spmd(nc, maps, core_ids=list(range(N_CORES)))
    _CACHE["maps"] = maps
    out = np.empty((32, 24, HW), np.float32)
    for c in range(N_CORES):
        out[c * B_PER_CORE:(c + 1) * B_PER_CORE] = res.results[c]["y_out"]
    return out.reshape(32, 24, 64, 64)


def _install_ntff_hook():
    import sys, types
    if "antenv.axon_hooks" in sys.modules:
        return
    state = {"hook": None}
    mod = types.ModuleType("antenv.axon_hooks")
    mod.set_axon_ntff_profile_hook = lambda h: state.__setitem__("hook", h)
    mod.get_axon_ntff_profile_hook = lambda: state["hook"]
    sys.modules["antenv.axon_hooks"] = mod
    import antenv
    antenv.axon_hooks = mod
    from trn_agent_boot.trn_boot import _ntff_profile_via_ctypes
    hook = _ntff_profile_via_ctypes("/opt/axon/libaxon_pjrt.so")
    if hook is not None:
        mod.set_axon_ntff_profile_hook(hook)


def profile():
    _install_ntff_hook()
    nc = _CACHE["nc"]
    res = run_bass_kernel_spmd(nc, _CACHE["maps"], core_ids=list(range(N_CORES)),
                               trace=True)
    return res.exec_time_ns, getattr(res, "instructions_and_trace", None)
